# revision 1
# baseline (speedup 1.0000x reference)
"""LocalLinear (per-position dense) Trainium2 kernel.

out[b, f, l] = sum_k xpad[b, f+k] * w[f, k, l] + bias[f, l]
  x: [256, 4096] f32, w: [4096, 64, 32] f32, bias: [4096, 32] f32
  out: [256, 4096, 32] f32

Strategy: fold-shard across 8 cores (512 folds each). Per group of 64
folds the einsum is one dense matmul [128u x 128b]^T @ [128u x 2048]
against a host-built banded (staircase) weight matrix: W[g, u, r, l] =
w[64g+r, u-r, l] for 0 <= u-r < 64, else 0. Matmuls run as float32r
(full PE rate, ~1e-4 absmax-relative error).
"""
import sys

if '/opt/trn_rl_repo' not in sys.path:
    sys.path.insert(0, '/opt/trn_rl_repo')

import numpy as np

import concourse.bass as bass
import concourse.tile as tile
from concourse import bacc, mybir
from concourse import bass_utils

B = 256
IN = 4096
KS = 64
L = 32
FOLD = 4096
NCORES = 8
FPC = FOLD // NCORES          # folds per core = 512
GPC = FPC // 64               # groups of 64 folds per core = 8
RL = 64 * L                   # 2048 free-dim columns per group

_DT = mybir.dt.float32r       # matmul operand dtype (f32 storage, fast path)
_cache = {}


def _build_nc(reps=1):
    nc = bacc.Bacc("TRN2", target_bir_lowering=False, debug=False)
    xt_d = nc.dram_tensor("xt", [GPC * 64 + 64, B], _DT, kind="ExternalInput")
    wb_d = nc.dram_tensor("wb", [GPC, 128, RL], _DT, kind="ExternalInput")
    out_d = nc.dram_tensor("out", [B, FPC, L], mybir.dt.float32,
                           kind="ExternalOutput")

    with tile.TileContext(nc) as tc:
        with (
            tc.tile_pool(name="xt", bufs=2) as xt_pool,
            tc.tile_pool(name="wb", bufs=2) as wb_pool,
            tc.tile_pool(name="ps", bufs=8, space="PSUM") as ps_pool,
            tc.tile_pool(name="ob", bufs=3) as ob_pool,
        ):
          for _rep in range(reps):
            for g in range(GPC):
                xt_t = xt_pool.tile([128, B], _DT)
                nc.sync.dma_start(xt_t[:], xt_d[64 * g: 64 * g + 128, :])
                wb_t = wb_pool.tile([128, RL], _DT)
                nc.sync.dma_start(wb_t[:], wb_d[g])
                for h in range(2):
                    ob = ob_pool.tile([128, 64, L], mybir.dt.float32)
                    for j in range(4):
                        ps = ps_pool.tile([128, 512], mybir.dt.float32)
                        nc.tensor.matmul(
                            ps[:],
                            xt_t[:, 128 * h: 128 * h + 128],
                            wb_t[:, 512 * j: 512 * j + 512],
                        )
                        dst = ob[:, 16 * j: 16 * j + 16, :]
                        if j % 2 == 0:
                            nc.vector.tensor_copy(dst, ps[:])
                        else:
                            nc.scalar.copy(dst, ps[:])
                    nc.sync.dma_start(
                        out_d[128 * h: 128 * h + 128,
                              64 * g: 64 * g + 64, :],
                        ob[:],
                    )
    nc.compile()
    return nc


def _host_prep(x, weight):
    # xt: padded transpose of x, [4160, 256]
    xt = np.zeros((FOLD + KS, B), np.float32)
    xt[:IN] = np.ascontiguousarray(x.T)
    # banded weights: W[g, u, 64r+... wb[g, u, r*L + l]
    G = FOLD // 64
    W = np.zeros((G, 128, 64, L), np.float32)
    wg = weight.reshape(G, 64, KS, L)
    for r in range(64):
        W[:, r:r + KS, r, :] = wg[:, r, :, :]
    W = W.reshape(G, 128, RL)
    return xt, W


def kernel(x, weight, bias):
    x = np.asarray(x, dtype=np.float32)
    weight = np.asarray(weight, dtype=np.float32)
    bias = np.asarray(bias, dtype=np.float32)

    if 'nc' not in _cache:
        _cache['nc'] = _build_nc()
    nc = _cache['nc']

    xt, W = _host_prep(x, weight)
    in_maps = []
    for c in range(NCORES):
        in_maps.append({
            "xt": np.ascontiguousarray(xt[FPC * c: FPC * c + FPC + KS]),
            "wb": np.ascontiguousarray(W[GPC * c: GPC * c + GPC]),
        })

    res = bass_utils.run_bass_kernel_spmd(
        nc, in_maps, core_ids=list(range(NCORES)), trace=False)

    out = np.concatenate([res.results[c]["out"] for c in range(NCORES)],
                         axis=1)
    if np.any(bias):
        out = out + bias[None, :, :]
    return out



# revision 2
# speedup vs baseline: 1.0480x; 1.0480x over previous
"""LocalLinear (per-position dense) Trainium2 kernel (optimized).

out[b, f, l] = sum_k xpad[b, f+k] * w[f, k, l] + bias[f, l]
  x: [256, 4096] f32, w: [4096, 64, 32] f32 -> out: [256, 4096, 32] f32

Fold-sharded across 8 cores (512 folds each); per 64-fold group the
einsum is a dense [128u x 128b]^T @ [128u x 2048] matmul against a
host-built banded weight matrix. All device I/O is bf16 (rel err
~4e-3 vs the 2e-2 gate); f32 upcast on host.

v4 = v2's fine DMA granularity (8x512KB weight loads so the first
matmul starts early, 8x1MB output stores) + v3's single big
[128, 2048] PSUM->SBUF cast per fold-group, alternating DVE/ACT.
"""
import sys

if '/opt/trn_rl_repo' not in sys.path:
    sys.path.insert(0, '/opt/trn_rl_repo')

import numpy as np
import ml_dtypes

import concourse.bass as bass
import concourse.tile as tile
from concourse import bacc, mybir
from concourse import bass_utils

B = 256
IN = 4096
KS = 64
L = 32
FOLD = 4096
NCORES = 8
FPC = FOLD // NCORES          # folds per core = 512
GPC = FPC // 64               # groups of 64 folds per core = 8
RL = 64 * L                   # 2048 free-dim columns per group

BF16 = mybir.dt.bfloat16
NPBF = ml_dtypes.bfloat16
_cache = {}


def _build_nc():
    nc = bacc.Bacc("TRN2", target_bir_lowering=False, debug=False)
    # xg[p, 256g+b] = xpad[b, 512c + 64g + p]  (pre-windowed on host)
    xg_d = nc.dram_tensor("xg", [128, GPC * B], BF16, kind="ExternalInput")
    wb_d = nc.dram_tensor("wb", [GPC, 128, RL], BF16, kind="ExternalInput")
    out_d = nc.dram_tensor("out", [B, FPC, L], BF16, kind="ExternalOutput")

    with tile.TileContext(nc) as tc:
        with (
            tc.tile_pool(name="xg", bufs=1) as xg_pool,
            tc.tile_pool(name="wb", bufs=GPC) as wb_pool,
            tc.tile_pool(name="ps", bufs=2, space="PSUM") as ps_pool,
            tc.tile_pool(name="ob", bufs=3) as ob_pool,
        ):
            xg_t = xg_pool.tile([128, GPC * B], BF16)
            nc.sync.dma_start(xg_t[:], xg_d[:])
            wb_ts = []
            for g in range(GPC):
                wb_t = wb_pool.tile([128, RL], BF16, tag="wb")
                nc.sync.dma_start(wb_t[:], wb_d[g])
                wb_ts.append(wb_t)

            cp = 0  # alternate PSUM->SBUF casts across DVE/ACT
            for h in range(2):
                for gp in range(GPC // 2):
                    ob = ob_pool.tile([128, 2, 64, L], BF16)
                    for s in range(2):
                        g = 2 * gp + s
                        lhsT = xg_t[:, B * g + 128 * h: B * g + 128 * h + 128]
                        ps = ps_pool.tile([128, 4 * 512], mybir.dt.float32)
                        for j in range(4):
                            nc.tensor.matmul(
                                ps[:, 512 * j: 512 * j + 512], lhsT,
                                wb_ts[g][:, 512 * j: 512 * j + 512])
                        dst = ob[:, s, :, :]
                        if cp % 2 == 0:
                            nc.vector.tensor_copy(dst, ps[:])
                        else:
                            nc.scalar.copy(dst, ps[:])
                        cp += 1
                    nc.sync.dma_start(
                        out_d[128 * h: 128 * h + 128,
                              128 * gp: 128 * gp + 128, :],
                        ob[:],
                    )
    nc.compile()
    return nc


def _host_prep(x, weight):
    # xt: padded transpose of x, [4160, 256], bf16
    xt = np.zeros((FOLD + KS, B), NPBF)
    xt[:IN] = np.ascontiguousarray(x.T)
    # banded weights: W[g, u, r*L + l] = w[64g+r, u-r, l] for 0<=u-r<64
    G = FOLD // 64
    W = np.zeros((G, 128, 64, L), NPBF)
    wg = weight.astype(NPBF).reshape(G, 64, KS, L)
    for r in range(64):
        W[:, r:r + KS, r, :] = wg[:, r, :, :]
    W = W.reshape(G, 128, RL)
    return xt, W


def _in_maps(x, weight):
    xt, W = _host_prep(x, weight)
    in_maps = []
    for c in range(NCORES):
        base = FPC * c
        xg = np.stack(
            [xt[base + 64 * g: base + 64 * g + 128] for g in range(GPC)],
            axis=1)  # [128, GPC, 256]
        in_maps.append({
            "xg": np.ascontiguousarray(xg.reshape(128, GPC * B)),
            "wb": np.ascontiguousarray(W[GPC * c: GPC * c + GPC]),
        })
    return in_maps


def kernel(x, weight, bias):
    x = np.asarray(x, dtype=np.float32)
    weight = np.asarray(weight, dtype=np.float32)
    bias = np.asarray(bias, dtype=np.float32)

    if 'nc' not in _cache:
        _cache['nc'] = _build_nc()
    nc = _cache['nc']

    in_maps = _in_maps(x, weight)
    res = bass_utils.run_bass_kernel_spmd(
        nc, in_maps, core_ids=list(range(NCORES)), trace=False)

    out = np.concatenate(
        [res.results[c]["out"].astype(np.float32) for c in range(NCORES)],
        axis=1)
    if np.any(bias):
        out = out + bias[None, :, :]
    return out
